# revision 1
# baseline (speedup 1.0000x reference)
"""CrossDomainGAT Trainium2 kernel.

Strategy (graph/data parallel, per sharding hint):
  - Destination nodes sharded across 8 cores (6250 dests/core, padded to 6272 =
    49 blocks x 128). Edges routed to the core owning the destination, so the
    per-edge softmax (over heads -- edge-local) and the scatter-add stay local.
  - Each core computes the full Q/V projections for all N source nodes itself
    (weights replicated; cheap on PE) and writes interleaved Q|V rows (bf16,
    512B/row) to a private HBM buffer.
  - Per dest-block of 128 nodes, edges are laid out dest-major: round r holds
    the r-th in-edge of each of the 128 dests (host pads per-dest edge lists;
    dests are degree-sorted so blocks have uniform round counts).  Q|V rows for
    a whole block are fetched with one batched dma_gather (512B elements at
    full DMA bandwidth).  K for the block is computed on the fly and stays
    partition-aligned with the dests, so no K gather is needed.
  - Per-edge math runs dest-major on DVE/ACT; the scatter-add accumulation runs
    on the TensorEngine as identity-matmul accumulation into PSUM.
  - Output projection + residual + LayerNorm fused per block; host un-permutes.

int16 gather indices only reach 32768 rows, so source rows are split into a
"lo" range (rq < LO_CUT) and a "hi" range; each dest's edges are grouped into
lo-rounds then hi-rounds and fetched with two gathers from two base offsets.
"""

import math
import numpy as np
import ml_dtypes

# ---------------------------------------------------------------- problem cfg
D = 128
H = 8
HD = 16
ALPHA = 0.2
LN_EPS = 1e-5

FULL_CFG = dict(
    N=50000,
    E=800000,
    NC=8,          # cores
    CHUNK=512,     # phase-1 node chunk
    RCHUNK=16,     # rounds per compute chunk
)


def _rq_map(n, chunk=512):
    """HBM row order for QV rows: within each phase-1 chunk of 512 nodes the
    SBUF tile [128p, 4j, 256] is written flat, so node n = s + j*128 + p lands
    at row s + p*4 + j.  Pure index math."""
    s = (n // chunk) * chunk
    t = n % chunk
    return s + (t % 128) * 4 + (t // 128)


def host_prep(x, edge_index, edge_attr, cfg):
    """Build per-core input arrays + uniform (SPMD) block metadata."""
    N, E, NC = cfg["N"], cfg["E"], cfg["NC"]
    CHUNK = cfg["CHUNK"]
    DPC = N // NC                      # dests per core
    NB = (DPC + 127) // 128            # dest blocks per core
    DPAD = NB * 128
    NCH = (N + CHUNK - 1) // CHUNK     # phase-1 chunks
    if NCH * CHUNK == N:
        NCH += 1                       # ensure zero slots for the hi sentinel
    NPAD = NCH * CHUNK                 # padded node slots

    row = np.asarray(edge_index[0], dtype=np.int64)
    col = np.asarray(edge_index[1], dtype=np.int64)
    ea = np.asarray(edge_attr, dtype=np.float32)
    x = np.asarray(x, dtype=np.float32)

    # rq row mapping for all source nodes
    rq_all = _rq_map(np.arange(NPAD, dtype=np.int64), CHUNK)
    # gather windows (int16 idx reaches 32768 rows; windows may overlap):
    #   QV rows: 0 = zero row, then row 1 + rq for rq in [0, NPAD)
    #   lo window = rows [0, LO_MAX+1]:       idx = 1 + rq, sentinel 0 (zero row)
    #   hi window = rows [1+HI_BASE, 1+NPAD): idx = rq - HI_BASE, sentinel = a
    #     zero node slot (n >= N) inside the hi window.
    LO_MAX = min(32766, NPAD - 1)      # max rq reachable via the lo window
    HI_BASE = max(0, NPAD - 32768)     # min rq reachable via the hi window
    assert NPAD > N, "need at least one padded node slot for the hi sentinel"
    zero_slots = np.arange(N, NPAD, dtype=np.int64)
    sent_rq = int(rq_all[zero_slots].max())
    assert sent_rq >= HI_BASE
    SENT_HI = sent_rq - HI_BASE
    assert 0 <= SENT_HI <= 32767
    HI_ROWS = NPAD - HI_BASE           # size of hi src window
    assert HI_ROWS <= 32768

    rq_e = rq_all[row]                 # rq of each edge's source
    # category: 0 = lo-only (rq < HI_BASE), 1 = flexible, 2 = hi-only
    cat = np.where(rq_e < HI_BASE, 0, np.where(rq_e <= LO_MAX, 1, 2)).astype(np.int8)
    idx_lo_e = (1 + rq_e).astype(np.int64)          # valid when cat <= 1
    idx_hi_e = (rq_e - HI_BASE).astype(np.int64)    # valid when cat >= 1

    core = col // DPC
    cl = col - core * DPC              # local dest id

    # ---- per-core degree sort + balanced lo/hi assignment
    perms = []
    RLc = np.zeros((NC, NB), dtype=np.int64)
    RHc = np.zeros((NC, NB), dtype=np.int64)
    per_core = []
    for c in range(NC):
        m = core == c
        clc = cl[m]
        catc = cat[m]
        Ld = np.bincount(clc[catc == 0], minlength=DPC)
        Fd = np.bincount(clc[catc == 1], minlength=DPC)
        Hd = np.bincount(clc[catc == 2], minlength=DPC)
        deg = Ld + Fd + Hd
        # balanced per-dest lo count: as close to deg/2 as the windows allow
        lo_n = np.clip((deg + 1) // 2, Ld, Ld + Fd)
        hi_n = deg - lo_n
        # group dests so blocks are homogeneous in (lo_n, hi_n): the per-block
        # round count is max(lo_n) + max(hi_n) over the block
        order = np.lexsort((hi_n, lo_n))
        perm = np.concatenate([order, np.full(DPAD - DPC, -1, np.int64)])
        inv = np.empty(DPC, dtype=np.int64)
        inv[order] = np.arange(DPC)
        perms.append(perm)
        q = inv[clc]                                # dest slot of each edge
        b = q // 128
        lo_s = np.concatenate([lo_n[order], np.zeros(DPAD - DPC, np.int64)])
        hi_s = np.concatenate([hi_n[order], np.zeros(DPAD - DPC, np.int64)])
        RLc[c] = lo_s.reshape(NB, 128).max(axis=1)
        RHc[c] = hi_s.reshape(NB, 128).max(axis=1)
        per_core.append(dict(m=m, q=q, b=b, catc=catc, lo_n=lo_n, clc=clc))

    RL = RLc.max(axis=0)               # uniform across cores
    RH = RHc.max(axis=0)
    RT = RL + RH
    n_rounds = int(RT.sum())

    # ---- per-core slot assignment + packed arrays
    # idx stream per block: lo rounds then hi rounds; stream position
    # i = r*128 + p; wrapped to [16, i//16] then tiled to 128 partitions.
    ea_off = np.concatenate([[0], np.cumsum(RT)])       # eattr col offsets (rounds)
    lo_off = np.concatenate([[0], np.cumsum(RL)])       # idx col offsets (rounds)
    hi_off = np.concatenate([[0], np.cumsum(RH)])
    tot_lo = int(RL.sum())
    tot_hi = int(RH.sum())

    idx_lo_arrs, idx_hi_arrs, ea_arrs, xd_arrs, xtd_arrs = [], [], [], [], []
    for c in range(NC):
        pc = per_core[c]
        m, q, b = pc["m"], pc["q"], pc["b"]
        p = q % 128
        # rank within dest, with lo-only edges first, then flexible, then
        # hi-only: the first lo_n[dest] edges go to the lo pass.
        key = q * 4 + pc["catc"].astype(np.int64)
        sort = np.argsort(key, kind="stable")
        qs = q[sort]
        starts = np.r_[0, np.flatnonzero(np.diff(qs)) + 1]
        counts = np.diff(np.r_[starts, len(qs)])
        rank_sorted = np.arange(len(qs)) - np.repeat(starts, counts)
        rank = np.empty(len(qs), np.int64)
        rank[sort] = rank_sorted
        lo_n_e = pc["lo_n"][pc["clc"]]             # per-edge lo split point
        il = rank < lo_n_e

        # lo slots
        strm_lo = np.zeros(tot_lo * 128, dtype=np.int16)   # sentinel 0
        el = np.flatnonzero(il)
        pos = (lo_off[b[el]] + rank[el]) * 128 + p[el]
        strm_lo[pos] = idx_lo_e[m][el].astype(np.int16)
        # hi slots
        strm_hi = np.full(tot_hi * 128, SENT_HI, dtype=np.int16)
        eh = np.flatnonzero(~il)
        posh = (hi_off[b[eh]] + (rank[eh] - lo_n_e[eh])) * 128 + p[eh]
        strm_hi[posh] = idx_hi_e[m][eh].astype(np.int16)
        # wrap to [16, cols] then tile to 128 partitions
        wl = strm_lo.reshape(-1, 16).T.copy()
        wh = strm_hi.reshape(-1, 16).T.copy()
        idx_lo_arrs.append(np.tile(wl, (8, 1)))
        idx_hi_arrs.append(np.tile(wh, (8, 1)))

        # eattr slot layout: [128 p, rounds, 16] with per-block lo rounds then
        # hi rounds at absolute round = ea_off[b] + r(lo) or + RL[b] + r(hi)
        eac = np.zeros((128, int(RT.sum()), 16), dtype=np.float32)
        r_abs = np.empty(len(qs), np.int64)
        r_abs[el] = ea_off[b[el]] + rank[el]
        r_abs[eh] = ea_off[b[eh]] + RL[b[eh]] + (rank[eh] - lo_n_e[eh])
        eac[p, r_abs] = ea[m]
        ea_arrs.append(eac.reshape(128, -1))

        # dest-side x (residual) and xT (K build), permuted to slot order
        perm = perms[c]
        xd = np.zeros((DPAD, D), dtype=np.float32)
        valid = perm >= 0
        xd[valid] = x[c * DPC + perm[valid]]
        xd_arrs.append(xd)
        xtd_arrs.append(np.ascontiguousarray(xd.T).astype(ml_dtypes.bfloat16))

    # xT for phase 1 (replicated)
    xpad = np.zeros((NPAD, D), dtype=np.float32)
    xpad[:N] = x
    xT = np.ascontiguousarray(xpad.T).astype(ml_dtypes.bfloat16)

    meta = dict(
        cfg=cfg, DPC=DPC, NB=NB, DPAD=DPAD, NCH=NCH, NPAD=NPAD,
        RL=RL.astype(int).tolist(), RH=RH.astype(int).tolist(),
        SENT_HI=SENT_HI, HI_ROWS=HI_ROWS, LO_MAX=LO_MAX, HI_BASE=HI_BASE,
        tot_lo=tot_lo, tot_hi=tot_hi, n_rounds=n_rounds,
        lo_off=lo_off.astype(int).tolist(), hi_off=hi_off.astype(int).tolist(),
        ea_off=ea_off.astype(int).tolist(),
    )
    arrs = dict(
        xT=xT, idx_lo=idx_lo_arrs, idx_hi=idx_hi_arrs, ea=ea_arrs,
        xd=xd_arrs, xtd=xtd_arrs, perms=perms,
    )
    return meta, arrs


# ------------------------------------------------------------------ weights
def host_weights(Wq, Wk, Wv, Wo, bo, gamma, beta):
    bf = ml_dtypes.bfloat16
    t = lambda W: np.ascontiguousarray(np.asarray(W, np.float32).T).astype(bf)
    rep = lambda v: np.tile(np.asarray(v, np.float32)[None, :], (128, 1))
    return dict(
        wq_t=t(Wq), wk_t=t(Wk), wv_t=t(Wv), wo_t=t(Wo),
        bo_b=rep(bo), gamma_b=rep(gamma), beta_b=rep(beta),
        ident=np.eye(128, dtype=np.float32).astype(bf),
    )


# ------------------------------------------------------------------ kernel IR
def build_nc(meta, debug=False, stage=None):
    import os as _os
    stage = stage or _os.environ.get("K_STAGE", "full")
    from contextlib import ExitStack
    import concourse.bacc as bacc
    import concourse.bass as bass
    import concourse.tile as tile
    from concourse import mybir

    cfg = meta["cfg"]
    NB, DPAD, NCH, NPAD = meta["NB"], meta["DPAD"], meta["NCH"], meta["NPAD"]
    RL, RH = meta["RL"], meta["RH"]
    CHUNK = cfg["CHUNK"]
    RCHUNK = cfg["RCHUNK"]
    LO_MAX, HI_BASE = meta["LO_MAX"], meta["HI_BASE"]
    HI_ROWS = meta["HI_ROWS"]
    tot_lo, tot_hi = meta["tot_lo"], meta["tot_hi"]
    n_rounds = meta["n_rounds"]
    lo_off, hi_off, ea_off = meta["lo_off"], meta["hi_off"], meta["ea_off"]
    RLMAX, RHMAX = max(RL), max(max(RH), 1)

    dt = mybir.dt
    AF = mybir.ActivationFunctionType
    AL = mybir.AluOpType

    nc = bacc.Bacc("TRN2", target_bir_lowering=False, debug=debug,
                   num_swdge_queues=4)

    # ---------- I/O ----------
    xT_d = nc.dram_tensor("xT", [128, NPAD], dt.bfloat16, kind="ExternalInput")
    xtd_d = nc.dram_tensor("xtd", [128, DPAD], dt.bfloat16, kind="ExternalInput")
    xd_d = nc.dram_tensor("xd", [DPAD, 128], dt.float32, kind="ExternalInput")
    idxlo_d = nc.dram_tensor("idx_lo", [128, tot_lo * 8], dt.int16, kind="ExternalInput")
    idxhi_d = nc.dram_tensor("idx_hi", [128, tot_hi * 8], dt.int16, kind="ExternalInput")
    ea_d = nc.dram_tensor("ea", [128, n_rounds * 16], dt.float32, kind="ExternalInput")
    wq_d = nc.dram_tensor("wq_t", [128, 128], dt.bfloat16, kind="ExternalInput")
    wk_d = nc.dram_tensor("wk_t", [128, 128], dt.bfloat16, kind="ExternalInput")
    wv_d = nc.dram_tensor("wv_t", [128, 128], dt.bfloat16, kind="ExternalInput")
    wo_d = nc.dram_tensor("wo_t", [128, 128], dt.bfloat16, kind="ExternalInput")
    bo_d = nc.dram_tensor("bo_b", [128, 128], dt.float32, kind="ExternalInput")
    ga_d = nc.dram_tensor("gamma_b", [128, 128], dt.float32, kind="ExternalInput")
    be_d = nc.dram_tensor("beta_b", [128, 128], dt.float32, kind="ExternalInput")
    id_d = nc.dram_tensor("ident", [128, 128], dt.bfloat16, kind="ExternalInput")
    y_d = nc.dram_tensor("y", [DPAD, 128], dt.float32, kind="ExternalOutput")

    # private HBM buffer of interleaved Q|V rows (bf16): row 0 zero, 1+rq
    qv_d = nc.dram_tensor("qv", [1 + NPAD, 256], dt.bfloat16)

    JC = CHUNK // 128  # sub-matmuls per phase-1 chunk

    with tile.TileContext(nc) as tc, ExitStack() as ctx:
        consts = ctx.enter_context(tc.tile_pool(name="consts", bufs=1))
        gpool = ctx.enter_context(tc.tile_pool(name="gath", bufs=2))
        mpool = ctx.enter_context(tc.tile_pool(name="meta", bufs=2))
        cpool = ctx.enter_context(tc.tile_pool(name="comp", bufs=3))
        spool = ctx.enter_context(tc.tile_pool(name="small", bufs=4))
        kpool = ctx.enter_context(tc.tile_pool(name="kblk", bufs=2))
        opool = ctx.enter_context(tc.tile_pool(name="outs", bufs=3))

        # ---------- constants ----------
        wq = consts.tile([128, 128], dt.bfloat16)
        wk = consts.tile([128, 128], dt.bfloat16)
        wvt = consts.tile([128, 128], dt.bfloat16)
        wo = consts.tile([128, 128], dt.bfloat16)
        bo = consts.tile([128, 128], dt.float32)
        ga = consts.tile([128, 128], dt.float32)
        be = consts.tile([128, 128], dt.float32)
        ident = consts.tile([128, 128], dt.bfloat16)
        epsT = consts.tile([128, 1], dt.float32)
        zrow = consts.tile([1, 256], dt.bfloat16)
        for dst, src in ((wq, wq_d), (wk, wk_d), (wvt, wv_d), (wo, wo_d),
                         (bo, bo_d), (ga, ga_d), (be, be_d), (ident, id_d)):
            nc.sync.dma_start(out=dst[:], in_=src[:])
        nc.vector.memset(epsT[:], LN_EPS)
        nc.vector.memset(zrow[:], 0.0)
        nc.sync.dma_start(out=qv_d[0:1, :], in_=zrow[:])

        # ---------- phase 1: Q|V rows to HBM ----------
        with tc.tile_pool(name="p1", bufs=3) as p1, \
             tc.tile_pool(name="p1ps", bufs=2, space="PSUM") as p1ps:
            for t in range(NCH):
                xt = p1.tile([128, CHUNK], dt.bfloat16, tag="xt")
                nc.sync.dma_start(out=xt[:], in_=xT_d[:, t * CHUNK:(t + 1) * CHUNK])
                psq = p1ps.tile([128, JC, 128], dt.float32, tag="psq")
                psv = p1ps.tile([128, JC, 128], dt.float32, tag="psv")
                for j in range(JC):
                    lhs = xt[:, j * 128:(j + 1) * 128]
                    nc.tensor.matmul(psq[:, j, :], lhs, wq[:], start=True, stop=True)
                    nc.tensor.matmul(psv[:, j, :], lhs, wvt[:], start=True, stop=True)
                qv = p1.tile([128, JC, 256], dt.bfloat16, tag="qvt")
                nc.scalar.copy(out=qv[:, :, 0:128], in_=psq[:])
                nc.scalar.copy(out=qv[:, :, 128:256], in_=psv[:])
                nc.sync.dma_start(
                    out=qv_d[1 + t * CHUNK:1 + (t + 1) * CHUNK, :]
                    .rearrange("(p j) e -> p j e", j=JC),
                    in_=qv[:],
                )

        qv_lo = qv_d[0:LO_MAX + 2, :]
        qv_hi = qv_d[1 + HI_BASE:1 + NPAD, :]

        psum = ctx.enter_context(tc.tile_pool(name="ps", bufs=1, space="PSUM"))
        accps = ctx.enter_context(tc.tile_pool(name="accps", bufs=2, space="PSUM"))

        # deferred-LN collection buffers (persist across the block loop)
        y2a = consts.tile([128, NB, 128], dt.float32)
        mva = consts.tile([128, NB, 2], dt.float32)

        # ---------- phase 2: per dest-block ----------
        for b in range(NB if stage != "p1" else 0):
            rl, rh = RL[b], RH[b]
            rt = rl + rh
            # K for this block: K = xtd_b.T @ wk (scaled 1/sqrt(HD) on copy)
            xtd = kpool.tile([128, 128], dt.bfloat16, tag="xtd")
            nc.sync.dma_start(out=xtd[:], in_=xtd_d[:, b * 128:(b + 1) * 128])
            kps = psum.tile([128, 128], dt.float32, tag="kps")
            nc.tensor.matmul(kps[:], xtd[:], wk[:], start=True, stop=True)
            kd = kpool.tile([128, 128], dt.bfloat16, tag="kd")
            nc.vector.tensor_scalar_mul(kd[:], kps[:], 1.0 / math.sqrt(HD))

            # gathers (whole block)
            glo = gpool.tile([128, RLMAX, 256], dt.bfloat16, tag="glo")
            ghi = gpool.tile([128, RHMAX, 256], dt.bfloat16, tag="ghi")
            if rl:
                ilo = mpool.tile([128, RLMAX * 8], dt.int16, tag="ilo")
                nc.sync.dma_start(out=ilo[:, :rl * 8],
                                  in_=idxlo_d[:, lo_off[b] * 8:(lo_off[b] + rl) * 8])
                nc.gpsimd.dma_gather(glo[:, :rl, :], qv_lo, ilo[:, :rl * 8],
                                     rl * 128, rl * 128, 256, elem_step=256,
                                     single_packet=False, queue_num=b % 4)
            if rh:
                ihi = mpool.tile([128, RHMAX * 8], dt.int16, tag="ihi")
                nc.sync.dma_start(out=ihi[:, :rh * 8],
                                  in_=idxhi_d[:, hi_off[b] * 8:(hi_off[b] + rh) * 8])
                nc.gpsimd.dma_gather(ghi[:, :rh, :], qv_hi, ihi[:, :rh * 8],
                                     rh * 128, rh * 128, 256, elem_step=256,
                                     single_packet=False, queue_num=b % 4)

            if stage == "gather":
                yg = opool.tile([128, 128], dt.float32, tag="yg")
                nc.vector.tensor_copy(out=yg[:], in_=glo[:, 0, 0:128])
                nc.sync.dma_start(out=y_d[b * 128:(b + 1) * 128, :], in_=yg[:])
                continue

            # edge weights for the whole block
            eat = mpool.tile([128, RLMAX + RHMAX, 16], dt.float32, tag="eat")
            nc.sync.dma_start(out=eat[:, :rt, :],
                              in_=ea_d[:, ea_off[b] * 16:(ea_off[b] + rt) * 16]
                              .rearrange("p (r s) -> p r s", s=16))
            # ew = sigmoid(sum ea) = 1 / (1 + exp(-sum)); Exp keeps the ACT
            # engine on a single LUT (no Sigmoid table swaps)
            easum = spool.tile([128, RLMAX + RHMAX], dt.float32, tag="easum")
            nc.vector.tensor_reduce(easum[:, :rt], eat[:, :rt, :],
                                    axis=mybir.AxisListType.X, op=AL.add,
                                    negate=True)
            een = spool.tile([128, RLMAX + RHMAX], dt.float32, tag="een")
            nc.scalar.activation(out=een[:, :rt], in_=easum[:, :rt], func=AF.Exp)
            ew1 = spool.tile([128, RLMAX + RHMAX], dt.float32, tag="ew1")
            nc.vector.tensor_scalar_add(ew1[:, :rt], een[:, :rt], 1.0)
            ew = spool.tile([128, RLMAX + RHMAX], dt.float32, tag="ew")
            nc.vector.reciprocal(out=ew[:, :rt], in_=ew1[:, :rt])

            # accumulator in PSUM via identity-matmul accumulation
            acc = accps.tile([128, 128], dt.float32, tag="acc")

            first_mm = True
            # chunks: lo rounds then hi rounds
            segs = []
            r0 = 0
            while r0 < rl:
                c = min(RCHUNK, rl - r0)
                segs.append((glo, r0, r0, c))
                r0 += c
            r0 = 0
            while r0 < rh:
                c = min(RCHUNK, rh - r0)
                segs.append((ghi, r0, rl + r0, c))
                r0 += c
            n_mm = sum(c for (_, _, _, c) in segs)
            mm_i = 0
            for (gt, gr, ar, c) in segs:
                g = gt[:, gr:gr + c, :]
                # prod = Qg * K (bcast over rounds)  [128, c, 128] bf16
                prod = cpool.tile([128, RCHUNK, 128], dt.bfloat16, tag="prod")
                kb = bass.AP(tensor=kd.tensor, offset=kd.offset,
                             ap=[list(kd.ap[0]), [0, c], [1, 128]])
                nc.vector.tensor_tensor(out=prod[:, :c, :], in0=g[:, :, 0:128],
                                        in1=kb, op=AL.mult)
                # head reduce via pairwise tree (tensor_reduce is 1x; dense
                # bf16 adds run 2x) -> [128, c, 8] f32
                p4 = prod[:, :c, :].rearrange("p c (h s) -> p c h s", s=16)
                t1_ = cpool.tile([128, RCHUNK, 8, 8], dt.bfloat16, tag="tr1")
                nc.vector.tensor_tensor(out=t1_[:, :c, :, :], in0=p4[:, :, :, 0:8],
                                        in1=p4[:, :, :, 8:16], op=AL.add)
                t2_ = cpool.tile([128, RCHUNK, 8, 4], dt.bfloat16, tag="tr2")
                nc.vector.tensor_tensor(out=t2_[:, :c, :, :], in0=t1_[:, :c, :, 0:4],
                                        in1=t1_[:, :c, :, 4:8], op=AL.add)
                t3_ = cpool.tile([128, RCHUNK, 8, 2], dt.bfloat16, tag="tr3")
                nc.vector.tensor_tensor(out=t3_[:, :c, :, :], in0=t2_[:, :c, :, 0:2],
                                        in1=t2_[:, :c, :, 2:4], op=AL.add)
                sraw = spool.tile([128, RCHUNK, 8], dt.float32, tag="sraw")
                nc.vector.tensor_tensor(out=sraw[:, :c, :], in0=t3_[:, :c, :, 0],
                                        in1=t3_[:, :c, :, 1], op=AL.add)
                # leaky relu: max(alpha*x, x)
                slr = spool.tile([128, RCHUNK, 8], dt.float32, tag="slr")
                nc.vector.scalar_tensor_tensor(out=slr[:, :c, :], in0=sraw[:, :c, :],
                                               scalar=ALPHA, in1=sraw[:, :c, :],
                                               op0=AL.mult, op1=AL.max)
                # * edge weight (bcast over heads)
                ewb = bass.AP(tensor=ew.tensor, offset=ew.offset + ar,
                              ap=[list(ew.ap[0]), [1, c], [0, 8]])
                sw = spool.tile([128, RCHUNK, 8], dt.float32, tag="sw")
                nc.vector.tensor_tensor(out=sw[:, :c, :], in0=slr[:, :c, :],
                                        in1=ewb, op=AL.mult)
                # exp (scores are small; no max-sub needed)
                esc = spool.tile([128, RCHUNK, 8], dt.float32, tag="esc")
                nc.scalar.activation(out=esc[:, :c, :], in_=sw[:, :c, :], func=AF.Exp)
                # sum over heads + reciprocal
                ses = spool.tile([128, RCHUNK], dt.float32, tag="ses")
                nc.vector.tensor_reduce(ses[:, :c], esc[:, :c, :],
                                        axis=mybir.AxisListType.X, op=AL.add)
                rec = spool.tile([128, RCHUNK], dt.float32, tag="rec")
                nc.vector.reciprocal(out=rec[:, :c], in_=ses[:, :c])
                # probs = esc * rec (bcast over heads) -> bf16
                rcb = bass.AP(tensor=rec.tensor, offset=rec.offset,
                              ap=[list(rec.ap[0]), [1, c], [0, 8]])
                probs = spool.tile([128, RCHUNK, 8], dt.bfloat16, tag="probs")
                nc.vector.tensor_tensor(out=probs[:, :c, :], in0=esc[:, :c, :],
                                        in1=rcb, op=AL.mult)
                # wv = Vg * probs (bcast 16 within head) [128, c, 128] bf16
                pb = bass.AP(tensor=probs.tensor, offset=probs.offset,
                             ap=[list(probs.ap[0]), [8, c], [1, 8], [0, 16]])
                wvt_t = cpool.tile([128, RCHUNK, 128], dt.bfloat16, tag="wv")
                nc.vector.tensor_tensor(out=wvt_t[:, :c, :], in0=g[:, :, 128:256],
                                        in1=pb, op=AL.mult)
                # accumulate: acc += I.T @ wv_r  (PE identity accumulation)
                for r in range(c):
                    nc.tensor.matmul(acc[:], ident[:], wvt_t[:, r, :],
                                     start=(mm_i == 0), stop=(mm_i == n_mm - 1),
                                     skip_group_check=True)
                    mm_i += 1

            if stage == "compute":
                yg = opool.tile([128, 128], dt.float32, tag="yg")
                nc.vector.tensor_copy(out=yg[:], in_=acc[:])
                nc.sync.dma_start(out=y_d[b * 128:(b + 1) * 128, :], in_=yg[:])
                continue

            # ---------- output stage (LN sqrt deferred + batched) ----------
            accs = opool.tile([128, 128], dt.bfloat16, tag="accs")
            nc.vector.tensor_copy(out=accs[:], in_=acc[:])
            accT = psum.tile([128, 128], dt.bfloat16, tag="accT")
            nc.tensor.transpose(accT[:], accs[:], ident[:])
            accTs = opool.tile([128, 128], dt.bfloat16, tag="accTs")
            nc.vector.tensor_copy(out=accTs[:], in_=accT[:])
            oproj = psum.tile([128, 128], dt.float32, tag="oproj")
            nc.tensor.matmul(oproj[:], accTs[:], wo[:], start=True, stop=True)

            xdt = opool.tile([128, 128], dt.float32, tag="xdt")
            nc.sync.dma_start(out=xdt[:], in_=xd_d[b * 128:(b + 1) * 128, :])
            y1 = opool.tile([128, 128], dt.float32, tag="y1")
            nc.vector.tensor_tensor(out=y1[:], in0=oproj[:], in1=xdt[:], op=AL.add)
            nc.vector.tensor_tensor(out=y2a[:, b, :], in0=y1[:], in1=bo[:],
                                    op=AL.add)
            st = spool.tile([128, 6], dt.float32, tag="st")
            nc.vector.bn_stats(out=st[:], in_=y2a[:, b, :])
            nc.vector.bn_aggr(out=mva[:, b, :], in_=st[:])

        if stage == "full":
            # batched LN: one sqrt + reciprocal for all blocks
            sd = consts.tile([128, NB], dt.float32)
            nc.scalar.activation(out=sd[:], in_=mva[:, :, 1], func=AF.Sqrt,
                                 bias=epsT[:])
            rstd = consts.tile([128, NB], dt.float32)
            nc.vector.reciprocal(out=rstd[:], in_=sd[:])
            for b in range(NB):
                t1 = opool.tile([128, 128], dt.float32, tag="t1")
                nc.vector.scalar_tensor_tensor(out=t1[:], in0=y2a[:, b, :],
                                               scalar=mva[:, b, 0:1], in1=ga[:],
                                               op0=AL.subtract, op1=AL.mult)
                yn = opool.tile([128, 128], dt.float32, tag="yn")
                nc.vector.scalar_tensor_tensor(out=yn[:], in0=t1[:],
                                               scalar=rstd[:, b:b + 1], in1=be[:],
                                               op0=AL.mult, op1=AL.add)
                nc.sync.dma_start(out=y_d[b * 128:(b + 1) * 128, :], in_=yn[:])

    nc.compile()
    return nc


# ------------------------------------------------------------------ runner
def _in_maps(meta, arrs, w):
    NC = meta["cfg"]["NC"]
    maps = []
    for c in range(NC):
        maps.append(dict(
            xT=np.ascontiguousarray(arrs["xT"]),
            xtd=np.ascontiguousarray(arrs["xtd"][c]),
            xd=np.ascontiguousarray(arrs["xd"][c]),
            idx_lo=np.ascontiguousarray(arrs["idx_lo"][c]),
            idx_hi=np.ascontiguousarray(arrs["idx_hi"][c]),
            ea=np.ascontiguousarray(arrs["ea"][c]),
            **{k: np.ascontiguousarray(v) for k, v in w.items()},
        ))
    return maps


def assemble(meta, arrs, results):
    cfg = meta["cfg"]
    N, NC, DPC = cfg["N"], cfg["NC"], meta["DPC"]
    out = np.empty((N, D), dtype=np.float32)
    for c in range(NC):
        yc = results[c]["y"]
        perm = arrs["perms"][c]
        valid = perm >= 0
        out[c * DPC + perm[valid]] = yc[:meta["DPAD"]][valid]
    return out


_CACHE = {}


def kernel(x, edge_index, edge_attr, Wq, Wk, Wv, Wo, bo, gamma, beta):
    cfg = FULL_CFG
    meta, arrs = host_prep(x, edge_index, edge_attr, cfg)
    w = host_weights(Wq, Wk, Wv, Wo, bo, gamma, beta)
    key = (tuple(meta["RL"]), tuple(meta["RH"]))
    if key not in _CACHE:
        _CACHE[key] = build_nc(meta)
    nc = _CACHE[key]
    from concourse.bass_utils import run_bass_kernel_spmd
    res = run_bass_kernel_spmd(nc, _in_maps(meta, arrs, w),
                               core_ids=list(range(cfg["NC"])))
    return assemble(meta, arrs, res.results)


if __name__ == "__main__":
    import reference
    inputs = {k: np.asarray(v) for k, v in reference.setup_inputs().items()}
    out = kernel(**inputs)
    exp = np.asarray(reference.reference(**reference.setup_inputs()))
    err = np.abs(out - exp).max() / max(np.abs(exp).max(), 1e-9)
    print("Relative error:", err)



# revision 8
# speedup vs baseline: 2.2923x; 2.2923x over previous
"""CrossDomainGAT Trainium2 kernel — gatherless edge-slot design.

Strategy (graph/data parallel per the sharding hint):
  - Destination nodes sharded across 8 cores (6250 dests/core, padded to
    6272 = 49 blocks x 128).  Edges are routed to the core owning the
    destination, so the per-edge softmax (over heads -- edge-local) and the
    scatter-add stay local.
  - NO on-device gather.  The host pre-gathers x^T into *edge-slot* order
    (dest-major rounds: slot s = r*128 + p holds the r-th in-edge of dest p
    of its block; dests are degree-sorted so blocks have uniform round
    counts).  The device computes Q and V *per edge slot* on the Tensor
    engine: per round, the 128-column x^T tile is the stationary operand and
    Wq^T / Wv^T stream through.  This replaces the baseline's
    dma_gather-based pipeline whose SWDGE descriptor generation saturated
    GpSimd (~8 ns/edge) and whose 512B-packet storm stalled DVE.
  - K for a block is computed once from the dest rows and broadcast across
    rounds via a stride-0 access pattern (dests are partition-aligned).
  - Per-edge math runs dest-major on DVE (ACT drains Q PSUM, Pool drains V
    PSUM); the scatter-add runs on the TensorEngine as identity-matmul
    accumulation into PSUM.
  - Output projection + residual + LayerNorm fused per block; LN sqrt is
    deferred and batched; host un-permutes.
  - Emission is software-pipelined one chunk ahead (stage1: DMA + QV
    matmuls + PSUM drains; stage2: edge math + accumulation) so the PE
    never sits behind the DVE chain of the same chunk.
"""

import math
import numpy as np
import ml_dtypes

# ---------------------------------------------------------------- problem cfg
D = 128
H = 8
HD = 16
ALPHA = 0.2
LN_EPS = 1e-5

FULL_CFG = dict(
    N=50000,
    E=800000,
    NC=8,          # cores
    RCHUNK=16,     # rounds per compute chunk
    GROUP=4,       # rounds per PSUM bank group
)


def host_prep(x, edge_index, edge_attr, cfg):
    """Route edges to dest cores, degree-sort dests into uniform blocks,
    and materialize x^T in edge-slot order (plus per-block dest tensors)."""
    N, E, NC = cfg["N"], cfg["E"], cfg["NC"]
    DPC = N // NC                      # dests per core
    NB = (DPC + 127) // 128            # dest blocks per core
    DPAD = NB * 128

    row = np.asarray(edge_index[0], dtype=np.int64)
    col = np.asarray(edge_index[1], dtype=np.int64)
    ea = np.asarray(edge_attr, dtype=np.float32)
    x = np.asarray(x, dtype=np.float32)
    bf = ml_dtypes.bfloat16

    core = col // DPC
    cl = col - core * DPC              # local dest id

    # ---- pass 1: per-core degree sort -> uniform per-block round counts
    perms, orders, degs = [], [], []
    Rc = np.zeros((NC, NB), dtype=np.int64)
    for c in range(NC):
        clc = cl[core == c]
        deg = np.bincount(clc, minlength=DPC)
        order = np.argsort(-deg, kind="stable")
        dpad = np.concatenate([deg[order], np.zeros(DPAD - DPC, np.int64)])
        Rc[c] = dpad.reshape(NB, 128).max(axis=1)
        perm = np.concatenate([order, np.full(DPAD - DPC, -1, np.int64)])
        perms.append(perm)
        orders.append(order)
        degs.append(deg)

    R = np.maximum(Rc.max(axis=0), 1)  # uniform across cores, >=1
    r_off = np.concatenate([[0], np.cumsum(R)])
    NR = int(R.sum())                  # total rounds per core
    S = NR * 128                       # edge slots per core

    # x^T padded with one zero column for pad slots
    xT = np.ascontiguousarray(x.T).astype(bf)
    xTpad = np.concatenate([xT, np.zeros((D, 1), dtype=bf)], axis=1)

    xdt_arrs, ea_arrs, xtd_arrs, xd_arrs = [], [], [], []
    for c in range(NC):
        m = core == c
        clc = cl[m]
        rowc = row[m]
        eac = ea[m]
        order = orders[c]
        inv = np.empty(DPC, dtype=np.int64)
        inv[order] = np.arange(DPC)
        q = inv[clc]                   # dest slot of each edge
        b = q // 128
        p = q % 128
        # rank within dest via stable sort on dest slot
        sort = np.argsort(q, kind="stable")
        qs = q[sort]
        starts = np.r_[0, np.flatnonzero(np.diff(qs)) + 1]
        counts = np.diff(np.r_[starts, len(qs)])
        rank_sorted = np.arange(len(qs)) - np.repeat(starts, counts)
        rank = np.empty(len(qs), np.int64)
        rank[sort] = rank_sorted

        s_idx = (r_off[b] + rank) * 128 + p
        src_col = np.full(S, N, dtype=np.int64)  # default: the zero column
        src_col[s_idx] = rowc
        xdt_arrs.append(np.ascontiguousarray(xTpad[:, src_col]))

        ea_l = np.zeros((S, 16), dtype=np.float32)
        ea_l[s_idx] = eac
        ea_arrs.append(np.ascontiguousarray(
            ea_l.reshape(NR, 128, 16).transpose(1, 0, 2).reshape(128, NR * 16)
        ).astype(bf))

        perm = perms[c]
        xd = np.zeros((DPAD, D), dtype=np.float32)
        valid = perm >= 0
        xd[valid] = x[c * DPC + perm[valid]]
        xd_arrs.append(xd)
        xtd_arrs.append(np.ascontiguousarray(xd.T).astype(bf))

    meta = dict(
        cfg=cfg, DPC=DPC, NB=NB, DPAD=DPAD,
        R=R.astype(int).tolist(), NR=NR, S=S,
        r_off=r_off.astype(int).tolist(),
        n_rounds=NR,
    )
    arrs = dict(
        xdt=xdt_arrs, ea=ea_arrs, xtd=xtd_arrs, xd=xd_arrs, perms=perms,
    )
    return meta, arrs


# ------------------------------------------------------------------ weights
def host_weights(Wq, Wk, Wv, Wo, bo, gamma, beta):
    bf = ml_dtypes.bfloat16
    t = lambda W: np.ascontiguousarray(np.asarray(W, np.float32).T).astype(bf)
    rep = lambda v: np.tile(np.asarray(v, np.float32)[None, :], (128, 1))
    return dict(
        wq_t=t(Wq), wk_t=t(Wk), wv_t=t(Wv), wo_t=t(Wo),
        bo_b=rep(bo), gamma_b=rep(gamma), beta_b=rep(beta),
        ident=np.eye(128, dtype=np.float32).astype(bf),
    )


# ------------------------------------------------------------------ kernel IR
def build_nc(meta, debug=False):
    from contextlib import ExitStack
    import concourse.bacc as bacc
    import concourse.bass as bass
    import concourse.tile as tile
    from concourse import mybir

    cfg = meta["cfg"]
    NB, DPAD = meta["NB"], meta["DPAD"]
    R = meta["R"]
    NR, S = meta["NR"], meta["S"]
    r_off = meta["r_off"]
    RCHUNK = cfg["RCHUNK"]
    GROUP = cfg["GROUP"]

    dt = mybir.dt
    AF = mybir.ActivationFunctionType
    AL = mybir.AluOpType

    nc = bacc.Bacc("TRN2", target_bir_lowering=False, debug=debug)

    # ---------- I/O ----------
    xdt_d = nc.dram_tensor("xdt", [128, S], dt.bfloat16, kind="ExternalInput")
    ea_d = nc.dram_tensor("ea", [128, NR * 16], dt.bfloat16, kind="ExternalInput")
    xtd_d = nc.dram_tensor("xtd", [128, DPAD], dt.bfloat16, kind="ExternalInput")
    xd_d = nc.dram_tensor("xd", [DPAD, 128], dt.float32, kind="ExternalInput")
    wq_d = nc.dram_tensor("wq_t", [128, 128], dt.bfloat16, kind="ExternalInput")
    wk_d = nc.dram_tensor("wk_t", [128, 128], dt.bfloat16, kind="ExternalInput")
    wv_d = nc.dram_tensor("wv_t", [128, 128], dt.bfloat16, kind="ExternalInput")
    wo_d = nc.dram_tensor("wo_t", [128, 128], dt.bfloat16, kind="ExternalInput")
    bo_d = nc.dram_tensor("bo_b", [128, 128], dt.float32, kind="ExternalInput")
    ga_d = nc.dram_tensor("gamma_b", [128, 128], dt.float32, kind="ExternalInput")
    be_d = nc.dram_tensor("beta_b", [128, 128], dt.float32, kind="ExternalInput")
    id_d = nc.dram_tensor("ident", [128, 128], dt.bfloat16, kind="ExternalInput")
    y_d = nc.dram_tensor("y", [DPAD, 128], dt.float32, kind="ExternalOutput")

    # chunk schedule: (block, r0_global, nr, first_in_block, last_in_block)
    chunks = []
    for b in range(NB):
        r0 = 0
        while r0 < R[b]:
            nr = min(RCHUNK, R[b] - r0)
            chunks.append((b, r_off[b] + r0, nr, r0 == 0, r0 + nr == R[b]))
            r0 += nr
    NCH = len(chunks)

    with tile.TileContext(nc) as tc, ExitStack() as ctx:
        consts = ctx.enter_context(tc.tile_pool(name="consts", bufs=1))
        xpool = ctx.enter_context(tc.tile_pool(name="xin", bufs=3))
        qvpool = ctx.enter_context(tc.tile_pool(name="qv", bufs=3))
        wvpool = ctx.enter_context(tc.tile_pool(name="wvp", bufs=3))
        spool = ctx.enter_context(tc.tile_pool(name="small", bufs=4))
        kpool = ctx.enter_context(tc.tile_pool(name="kblk", bufs=2))
        opool = ctx.enter_context(tc.tile_pool(name="outs", bufs=3))
        # PSUM budget (8 banks x 2KB): psq/psv tags 2 bufs each = 4 banks,
        # acc 1 bank, kps/accT/oproj 1 buf each = 3 banks.
        psqv = ctx.enter_context(tc.tile_pool(name="psqv", bufs=2, space="PSUM"))
        psacc = ctx.enter_context(tc.tile_pool(name="psacc", bufs=1, space="PSUM"))
        psmisc = ctx.enter_context(tc.tile_pool(name="psmisc", bufs=1, space="PSUM"))

        # ---------- constants ----------
        wq = consts.tile([128, 128], dt.bfloat16)
        wk = consts.tile([128, 128], dt.bfloat16)
        wvt = consts.tile([128, 128], dt.bfloat16)
        wo = consts.tile([128, 128], dt.bfloat16)
        bo = consts.tile([128, 128], dt.float32)
        ga = consts.tile([128, 128], dt.float32)
        be = consts.tile([128, 128], dt.float32)
        ident = consts.tile([128, 128], dt.bfloat16)
        epsT = consts.tile([128, 1], dt.float32)
        for dst, src in ((wq, wq_d), (wk, wk_d), (wvt, wv_d), (wo, wo_d),
                         (bo, bo_d), (ga, ga_d), (be, be_d), (ident, id_d)):
            nc.sync.dma_start(out=dst[:], in_=src[:])
        nc.vector.memset(epsT[:], LN_EPS)

        # deferred-LN collection buffers (persist across the block loop)
        y2a = consts.tile([128, NB, 128], dt.float32)
        mva = consts.tile([128, NB, 2], dt.float32)

        # per-chunk state carried from stage1 to stage2
        state = [None] * NCH
        # per-block state
        kd_t = [None] * NB
        acc_t = [None] * NB
        xd_t = [None] * NB
        mm_done = [0] * NB

        def stage1(k):
            b, g0, nr, first, last = chunks[k]
            st = {}
            if first:
                # K for this block: K = xtd_b.T @ wk, scaled 1/sqrt(HD)
                xtd = kpool.tile([128, 128], dt.bfloat16, tag="xtd")
                nc.sync.dma_start(out=xtd[:], in_=xtd_d[:, b * 128:(b + 1) * 128])
                kps = psmisc.tile([128, 128], dt.float32, tag="kps")
                nc.tensor.matmul(kps[:], xtd[:], wk[:], start=True, stop=True)
                kd = kpool.tile([128, 128], dt.bfloat16, tag="kd")
                nc.vector.tensor_scalar_mul(kd[:], kps[:], 1.0 / math.sqrt(HD))
                kd_t[b] = kd
                acc_t[b] = psacc.tile([128, 128], dt.float32, tag="acc",
                                      name="acc")
                xdt_ = opool.tile([128, 128], dt.float32, tag="xdt")
                nc.sync.dma_start(out=xdt_[:], in_=xd_d[b * 128:(b + 1) * 128, :])
                xd_t[b] = xdt_
                mm_done[b] = 0

            # input slices
            xt = xpool.tile([128, RCHUNK * 128], dt.bfloat16, tag="xt")
            nc.sync.dma_start(out=xt[:, :nr * 128],
                              in_=xdt_d[:, g0 * 128:(g0 + nr) * 128])
            eat = xpool.tile([128, RCHUNK, 16], dt.bfloat16, tag="eat")
            nc.sync.dma_start(out=eat[:, :nr, :],
                              in_=ea_d[:, g0 * 16:(g0 + nr) * 16]
                              .rearrange("p (r s) -> p r s", s=16))

            # Q projection per round; drain PSUM via ACT.  (V runs in stage2
            # so its PSUM drain fuses into the probs multiply on DVE.)
            qs = qvpool.tile([128, RCHUNK, 128], dt.bfloat16, tag="qs")
            r0 = 0
            while r0 < nr:
                gn = min(GROUP, nr - r0)
                psq = psqv.tile([128, GROUP, 128], dt.float32, tag="psq")
                for r in range(gn):
                    lhs = xt[:, (r0 + r) * 128:(r0 + r + 1) * 128]
                    nc.tensor.matmul(psq[:, r, :], lhs, wq[:], start=True, stop=True)
                nc.scalar.copy(out=qs[:, r0:r0 + gn, :], in_=psq[:, :gn, :])
                r0 += gn
            st["xt"], st["eat"], st["qs"] = xt, eat, qs
            state[k] = st

        def stage2(k):
            b, g0, nr, first, last = chunks[k]
            st = state[k]
            xt, eat, qs = st["xt"], st["eat"], st["qs"]
            kd = kd_t[b]
            acc = acc_t[b]
            c = nr

            # edge weight: ew = sigmoid(sum ea) via Exp-only path
            easum = spool.tile([128, RCHUNK], dt.float32, tag="easum")
            nc.vector.tensor_reduce(easum[:, :c], eat[:, :c, :],
                                    axis=mybir.AxisListType.X, op=AL.add,
                                    negate=True)
            een = spool.tile([128, RCHUNK], dt.float32, tag="een")
            nc.scalar.activation(out=een[:, :c], in_=easum[:, :c], func=AF.Exp)
            ew1 = spool.tile([128, RCHUNK], dt.float32, tag="ew1")
            nc.vector.tensor_scalar_add(ew1[:, :c], een[:, :c], 1.0)
            ew = spool.tile([128, RCHUNK], dt.float32, tag="ew")
            nc.vector.reciprocal(out=ew[:, :c], in_=ew1[:, :c])

            # prod = Q * K (K broadcast over rounds)
            prod = qvpool.tile([128, RCHUNK, 128], dt.bfloat16, tag="prod")
            kb = bass.AP(tensor=kd.tensor, offset=kd.offset,
                         ap=[list(kd.ap[0]), [0, c], [1, 128]])
            nc.vector.tensor_tensor(out=prod[:, :c, :], in0=qs[:, :c, :],
                                    in1=kb, op=AL.mult)
            # head reduce via pairwise tree
            p4 = prod[:, :c, :].rearrange("p c (h s) -> p c h s", s=16)
            t1_ = wvpool.tile([128, RCHUNK, 8, 8], dt.bfloat16, tag="tr1")
            nc.vector.tensor_tensor(out=t1_[:, :c, :, :], in0=p4[:, :, :, 0:8],
                                    in1=p4[:, :, :, 8:16], op=AL.add)
            t2_ = spool.tile([128, RCHUNK, 8, 4], dt.bfloat16, tag="tr2")
            nc.vector.tensor_tensor(out=t2_[:, :c, :, :], in0=t1_[:, :c, :, 0:4],
                                    in1=t1_[:, :c, :, 4:8], op=AL.add)
            t3_ = spool.tile([128, RCHUNK, 8, 2], dt.bfloat16, tag="tr3")
            nc.vector.tensor_tensor(out=t3_[:, :c, :, :], in0=t2_[:, :c, :, 0:2],
                                    in1=t2_[:, :c, :, 2:4], op=AL.add)
            sraw = spool.tile([128, RCHUNK, 8], dt.float32, tag="sraw")
            nc.vector.tensor_tensor(out=sraw[:, :c, :], in0=t3_[:, :c, :, 0],
                                    in1=t3_[:, :c, :, 1], op=AL.add)
            # leaky relu: max(alpha*x, x)
            slr = spool.tile([128, RCHUNK, 8], dt.float32, tag="slr")
            nc.vector.scalar_tensor_tensor(out=slr[:, :c, :], in0=sraw[:, :c, :],
                                           scalar=ALPHA, in1=sraw[:, :c, :],
                                           op0=AL.mult, op1=AL.max)
            # * edge weight (bcast over heads)
            ewb = bass.AP(tensor=ew.tensor, offset=ew.offset,
                          ap=[list(ew.ap[0]), [1, c], [0, 8]])
            sw = spool.tile([128, RCHUNK, 8], dt.float32, tag="sw")
            nc.vector.tensor_tensor(out=sw[:, :c, :], in0=slr[:, :c, :],
                                    in1=ewb, op=AL.mult)
            # exp (scores are small; no max-sub needed)
            esc = spool.tile([128, RCHUNK, 8], dt.float32, tag="esc")
            nc.scalar.activation(out=esc[:, :c, :], in_=sw[:, :c, :], func=AF.Exp)
            # sum over heads + reciprocal
            ses = spool.tile([128, RCHUNK], dt.float32, tag="ses")
            nc.vector.tensor_reduce(ses[:, :c], esc[:, :c, :],
                                    axis=mybir.AxisListType.X, op=AL.add)
            rec = spool.tile([128, RCHUNK], dt.float32, tag="rec")
            nc.vector.reciprocal(out=rec[:, :c], in_=ses[:, :c])
            # probs = esc * rec (bcast over heads) -> bf16
            rcb = bass.AP(tensor=rec.tensor, offset=rec.offset,
                          ap=[list(rec.ap[0]), [1, c], [0, 8]])
            probs = spool.tile([128, RCHUNK, 8], dt.bfloat16, tag="probs")
            nc.vector.tensor_tensor(out=probs[:, :c, :], in0=esc[:, :c, :],
                                    in1=rcb, op=AL.mult)
            # V projection per group; PSUM drain fused into the probs
            # multiply: wv = V_psum * probs (bcast 16 within head) -> bf16
            wvt_t = wvpool.tile([128, RCHUNK, 128], dt.bfloat16, tag="wv")
            n_mm = R[b]
            r0 = 0
            while r0 < c:
                gn = min(GROUP, c - r0)
                psv = psqv.tile([128, GROUP, 128], dt.float32, tag="psv")
                for r in range(gn):
                    lhs = xt[:, (r0 + r) * 128:(r0 + r + 1) * 128]
                    nc.tensor.matmul(psv[:, r, :], lhs, wvt[:], start=True, stop=True)
                pb = bass.AP(tensor=probs.tensor, offset=probs.offset + r0 * 8,
                             ap=[list(probs.ap[0]), [8, gn], [1, 8], [0, 16]])
                nc.vector.tensor_tensor(out=wvt_t[:, r0:r0 + gn, :],
                                        in0=psv[:, :gn, :], in1=pb, op=AL.mult)
                r0 += gn
            # accumulate: acc += I.T @ wv_r
            for r in range(c):
                i = mm_done[b]
                nc.tensor.matmul(acc[:], ident[:], wvt_t[:, r, :],
                                 start=(i == 0), stop=(i == n_mm - 1),
                                 skip_group_check=True)
                mm_done[b] += 1

            if last:
                # ---------- output stage (LN sqrt deferred + batched) ----
                accs = opool.tile([128, 128], dt.bfloat16, tag="accs")
                nc.vector.tensor_copy(out=accs[:], in_=acc[:])
                accT = psmisc.tile([128, 128], dt.bfloat16, tag="accT")
                nc.tensor.transpose(accT[:], accs[:], ident[:])
                accTs = opool.tile([128, 128], dt.bfloat16, tag="accTs")
                nc.scalar.copy(out=accTs[:], in_=accT[:])
                oproj = psmisc.tile([128, 128], dt.float32, tag="oproj")
                nc.tensor.matmul(oproj[:], accTs[:], wo[:], start=True, stop=True)

                y1 = opool.tile([128, 128], dt.float32, tag="y1")
                nc.vector.tensor_tensor(out=y1[:], in0=oproj[:], in1=xd_t[b][:],
                                        op=AL.add)
                nc.vector.tensor_tensor(out=y2a[:, b, :], in0=y1[:], in1=bo[:],
                                        op=AL.add)
                stt = spool.tile([128, 6], dt.float32, tag="st")
                nc.vector.bn_stats(out=stt[:], in_=y2a[:, b, :])
                nc.vector.bn_aggr(out=mva[:, b, :], in_=stt[:])

        # software-pipelined emission: stage1 one chunk ahead of stage2
        stage1(0)
        for k in range(NCH):
            if k + 1 < NCH:
                stage1(k + 1)
            stage2(k)

        # batched LN: one sqrt + reciprocal for all blocks
        sd = consts.tile([128, NB], dt.float32)
        nc.scalar.activation(out=sd[:], in_=mva[:, :, 1], func=AF.Sqrt,
                             bias=epsT[:])
        rstd = consts.tile([128, NB], dt.float32)
        nc.vector.reciprocal(out=rstd[:], in_=sd[:])
        for b in range(NB):
            t1 = opool.tile([128, 128], dt.float32, tag="t1")
            nc.vector.scalar_tensor_tensor(out=t1[:], in0=y2a[:, b, :],
                                           scalar=mva[:, b, 0:1], in1=ga[:],
                                           op0=AL.subtract, op1=AL.mult)
            yn = opool.tile([128, 128], dt.float32, tag="yn")
            nc.vector.scalar_tensor_tensor(out=yn[:], in0=t1[:],
                                           scalar=rstd[:, b:b + 1], in1=be[:],
                                           op0=AL.mult, op1=AL.add)
            nc.sync.dma_start(out=y_d[b * 128:(b + 1) * 128, :], in_=yn[:])

    nc.compile()
    return nc


# ------------------------------------------------------------------ runner
def _in_maps(meta, arrs, w):
    NC = meta["cfg"]["NC"]
    maps = []
    for c in range(NC):
        maps.append(dict(
            xdt=np.ascontiguousarray(arrs["xdt"][c]),
            ea=np.ascontiguousarray(arrs["ea"][c]),
            xtd=np.ascontiguousarray(arrs["xtd"][c]),
            xd=np.ascontiguousarray(arrs["xd"][c]),
            **{k: np.ascontiguousarray(v) for k, v in w.items()},
        ))
    return maps


def assemble(meta, arrs, results):
    cfg = meta["cfg"]
    N, NC, DPC = cfg["N"], cfg["NC"], meta["DPC"]
    out = np.empty((N, D), dtype=np.float32)
    for c in range(NC):
        yc = results[c]["y"]
        perm = arrs["perms"][c]
        valid = perm >= 0
        out[c * DPC + perm[valid]] = yc[:meta["DPAD"]][valid]
    return out


_CACHE = {}


def kernel(x, edge_index, edge_attr, Wq, Wk, Wv, Wo, bo, gamma, beta):
    cfg = FULL_CFG
    meta, arrs = host_prep(x, edge_index, edge_attr, cfg)
    w = host_weights(Wq, Wk, Wv, Wo, bo, gamma, beta)
    key = tuple(meta["R"])
    if key not in _CACHE:
        _CACHE[key] = build_nc(meta)
    nc = _CACHE[key]
    from concourse.bass_utils import run_bass_kernel_spmd
    res = run_bass_kernel_spmd(nc, _in_maps(meta, arrs, w),
                               core_ids=list(range(cfg["NC"])))
    return assemble(meta, arrs, res.results)


if __name__ == "__main__":
    import reference
    inputs = {k: np.asarray(v) for k, v in reference.setup_inputs().items()}
    out = kernel(**inputs)
    exp = np.asarray(reference.reference(**reference.setup_inputs()))
    err = np.abs(out - exp).max() / max(np.abs(exp).max(), 1e-9)
    print("Relative error:", err)
